# revision 28
# baseline (speedup 1.0000x reference)
"""NeuroPlasticLite Trainium2 kernel (8-core data-parallel over batch).

Layouts (per core, batch shard BS=64, rows r = (b, n), 16384 rows):
  R-layout: SBUF [128 part = nlo, free = fc] with fc = nhi*64 + b,
            n = nhi*128 + nlo.  x free = fc*32 + d.
  T-layout (gT/zT): [128 part = (a, j) = a*16+j, free = O*128 + nlo],
            block O covers fc = 8*O + a.

Scaled basis: xs_t = c1^{-t} x_t, so each step is a pure accumulation
  xs += c1^{-(t+1)} * (DT*h + V)
with the c1 scale folded into per-step consts (identS / bdcs).  Norm uses
act Sqrt(scale=c1^{2t} * nsq + eps).  Final x = c1^20 * xs (host side).

Per step:
  xsq = xs*xs (DVE fp16 2x), nsq = reduce_32 (DVE), nrm = sqrt (Act),
  a = tanh (Act fp16), synT = a @ W blocks (PE, transposed out),
  zT = W1d-expand @ synT (PE), g = gelu(zT + b1) (Act, one op per half),
  s = identS@V + gT @ bdcs (PE, PSUM), xs += s (DVE/Pool TT-add).
"""

import os
from contextlib import ExitStack

import numpy as np

N, D, KF, KN = 256, 32, 16, 50
GAMMA, LAM_A, DT, STEPS = 0.1, 0.95, 0.05, 20
B, UIN = 512, 128
NCORES = 8
BS = B // NCORES          # 64 batch rows per core
R = BS * N                # 16384 rows per core
C1 = 1.0 - DT * GAMMA     # 0.995
EPS = 1e-12

_cache = {}


def _host_prep(features, bias, w_in, b_in, sig_w1, sig_b1, sig_w2, sig_b2):
    """All tiny, replicated tensors (fp16 on-device consts)."""
    f16 = np.float16
    f = features / np.linalg.norm(features, axis=1, keepdims=True)
    sim = f @ f.T                                   # [N, N]
    idx = np.argsort(-sim, axis=1, kind="stable")[:, :KN]        # [N, KN]
    vals = np.take_along_axis(sim, idx, axis=1)                  # [N, KN]
    W = np.zeros((N, N), np.float32)                             # W[m, n]
    np.add.at(W, (idx, np.arange(N)[:, None]), vals)

    # synT-mm rhs blocks: wt[:, (mh*2+nh)*128 + nlo] [mlo, nlo]
    wt = np.concatenate(
        [W[mh * 128:(mh + 1) * 128, nh * 128:(nh + 1) * 128]
         for mh in (0, 1) for nh in (0, 1)], axis=1,
    ).astype(f16)                                                # [128, 512]

    winTc = (DT * w_in.T).astype(f16)                            # [128, 32]
    # baddSm[p=nlo, nhi*32+d] = DT*(bias[n,d] + b_in[d] + sig_b2[d])
    badd = DT * (bias + b_in[None, :] + sig_b2[None, :])         # [256, 32]
    baddSm = np.concatenate([badd[0:128, :], badd[128:256, :]],
                            axis=1).astype(f16)                  # [128, 64]

    # zT-mm lhsT: W1d[fc, O*128 + a*16+j] = (fc == 8*O+a) * w1[j]
    W1d = np.zeros((128, 16 * 128), np.float32)
    w1 = sig_w1[:, 0].astype(np.float32)                         # [16]
    for O in range(16):
        for a in range(8):
            fc = 8 * O + a
            W1d[fc, O * 128 + a * 16:O * 128 + (a + 1) * 16] = w1
    W1d = W1d.astype(f16)                                        # [128, 2048]

    # gelu bias per partition (a,j): b1v[a*16+j] = sig_b1[j]
    b1v = np.tile(sig_b1.astype(np.float32), 8)[:, None]         # [128, 1] f32

    # bd blocks with per-step scale: bdcs[:, t*256 + a'*32+d]
    #   [a*16+j, a'*32+d] = (a==a') * c1^{-(t+1)} * DT * w2[d, j]
    bd0 = np.zeros((128, 256), np.float32)
    for a in range(8):
        bd0[a * 16:(a + 1) * 16, a * 32:(a + 1) * 32] = DT * sig_w2.T
    bdcs = np.concatenate(
        [bd0 * (C1 ** -(t + 1)) for t in range(STEPS)], axis=1
    ).astype(f16)                                                # [128, 5120]

    # identS[:, t*128:(t+1)*128] = c1^{-(t+1)} * I
    ident = np.eye(128, dtype=np.float32)
    identS = np.concatenate(
        [ident * (C1 ** -(t + 1)) for t in range(STEPS)], axis=1
    ).astype(f16)                                                # [128, 2560]

    return wt, winTc, baddSm, W1d, b1v.astype(np.float32), bdcs, identS


def build_nc(n_cores):
    import concourse.bacc as bacc
    import concourse.tile as tile
    from concourse import mybir

    f32 = mybir.dt.float32
    i32 = mybir.dt.int32
    f16 = mybir.dt.float16
    AF = mybir.ActivationFunctionType
    OP = mybir.AluOpType
    AX = mybir.AxisListType

    nc = bacc.Bacc("TRN2", target_bir_lowering=False, debug=False,
                   num_devices=n_cores)
    u_t = nc.declare_dram_parameter("u_t", [R, UIN], f16, isOutput=False)
    wt_d = nc.declare_dram_parameter("wt", [128, 512], f16, isOutput=False)
    winT_d = nc.declare_dram_parameter("winTc", [128, 32], f16, isOutput=False)
    badd_d = nc.declare_dram_parameter("baddSm", [128, 64], f16, isOutput=False)
    w1d_d = nc.declare_dram_parameter("W1d", [128, 2048], f16, isOutput=False)
    b1v_d = nc.declare_dram_parameter("b1v", [128, 1], f32, isOutput=False)
    bdcs_d = nc.declare_dram_parameter("bdcs", [128, 256 * STEPS], f16,
                                       isOutput=False)
    idS_d = nc.declare_dram_parameter("identS", [128, 128 * STEPS], f16,
                                      isOutput=False)
    xout = nc.declare_dram_parameter("xout", [128, 4096], f16, isOutput=True)

    with tile.TileContext(nc) as tc:
        with ExitStack() as ctx:
            cpool = ctx.enter_context(tc.tile_pool(name="consts", bufs=1))
            wt = cpool.tile([128, 512], f16)
            nc.sync.dma_start(wt[:], wt_d[:])
            winTc = cpool.tile([128, 32], f16)
            nc.sync.dma_start(winTc[:], winT_d[:])
            baddSm = cpool.tile([128, 64], f16)
            nc.sync.dma_start(baddSm[:], badd_d[:])
            W1d = cpool.tile([128, 2048], f16)
            nc.sync.dma_start(W1d[:], w1d_d[:])
            b1v = cpool.tile([128, 1], f32)
            nc.sync.dma_start(b1v[:], b1v_d[:])
            bdcs = cpool.tile([128, 256 * STEPS], f16)
            nc.sync.dma_start(bdcs[:], bdcs_d[:])
            identS = cpool.tile([128, 128 * STEPS], f16)
            nc.sync.dma_start(identS[:], idS_d[:])
            magic = cpool.tile([128, 1], mybir.dt.int32)
            nc.vector.memset(magic[:], 0x5F3759DF)

            spool = ctx.enter_context(tc.tile_pool(name="state", bufs=1))
            xs = [spool.tile([128, 2048], f16, name=f"xs{i}") for i in range(2)]
            V_sb = [spool.tile([128, 2048], f16, name=f"V{i}") for i in range(2)]
            gT = [spool.tile([128, 1024], f16, name=f"gT{i}") for i in range(2)]
            gR = [spool.tile([128, 1024], f16, name=f"gR{i}") for i in range(2)]
            uT = spool.tile([128, R], f16)

            nc.vector.memset(xs[0][:], 0.0)
            nc.vector.memset(xs[1][:], 0.0)

            # ---------- Phase A: transposed u load + u_proj -> V ----------
            with ExitStack() as actx:
                vpp = actx.enter_context(
                    tc.tile_pool(name="vp", bufs=2, space="PSUM"))
                for ch in range(4):
                    nc.sync.dma_start_transpose(
                        uT[:, ch * 4096:(ch + 1) * 4096],
                        u_t[ch * 4096:(ch + 1) * 4096, :])
                for q in range(8):                  # V bank q: fc 16q..16q+16
                    vp = vpp.tile([128, 512], f32, tag=f"v{q % 2}")
                    for k in range(16):
                        fc = 16 * q + k
                        nhi, b = fc // 64, fc % 64
                        r0 = b * 256 + nhi * 128
                        nc.tensor.matmul(
                            vp[:, k * 32:(k + 1) * 32],
                            uT[:, r0:r0 + 128], winTc[:],
                            start=(k % 16 == 0), stop=(k % 16 == 15),
                            skip_group_check=True)
                    nhi = (16 * q) // 64
                    bsl = baddSm[:, nhi * 32:(nhi + 1) * 32]
                    brd = bsl.unsqueeze(1).broadcast_to((128, 16, 32))
                    st, g, sub = (q // 2) % 2, q // 4, (q % 2) * 512
                    nc.vector.tensor_tensor(
                        V_sb[st][:, g * 1024 + sub:g * 1024 + sub + 512]
                        .rearrange("p (s d) -> p s d", d=32),
                        vp[:].rearrange("p (s d) -> p s d", d=32),
                        brd, op=OP.add)

            # ---------- Phase B: 20 steps ----------
            lpool = ctx.enter_context(tc.tile_pool(name="loop", bufs=2))
            synp = ctx.enter_context(
                tc.tile_pool(name="synp", bufs=1, space="PSUM"))
            zpp = ctx.enter_context(
                tc.tile_pool(name="zp", bufs=1, space="PSUM"))
            spp = ctx.enter_context(
                tc.tile_pool(name="sp", bufs=1, space="PSUM"))
            xpp = ctx.enter_context(
                tc.tile_pool(name="xp", bufs=1, space="PSUM"))

            # pre-zero synT psum tiles (unwritten partitions must be 0)
            synT_all = synp.tile([128, 256], f32, tag="synT", name="synT")
            synT_ps = [synT_all[:, 0:128], synT_all[:, 128:256]]
            nc.vector.memset(synT_all[:], 0.0)
            # PSUM-resident xs accumulators for the g=0 (nhi=0) half
            xs0p = [xpp.tile([128, 1024], f32, tag=f"xs0p{s}", name=f"xs0p{s}")
                    for s in (0, 1)]

            a_sb = [None, None]
            synT_sb = [None, None]

            def emit_norm(s, t):
                """nsq/sqrt/tanh -> a_sb[s] for step t (reads state of t)."""
                a_sb[s] = lpool.tile([128, 64], f16, tag=f"a{s}", name=f"a{s}")
                if t == 0:
                    nc.vector.memset(a_sb[s][:], 0.0)
                    return
                xsq = lpool.tile([128, 2048], f16, tag=f"xsq{s}", name=f"xsq{s}")
                # g0 half from PSUM (Act), g1 half from SBUF
                nc.scalar.activation(xsq[:, 0:1024], xs0p[s][:], AF.Square)
                if s == 0:
                    nc.vector.tensor_tensor(
                        xsq[:, 1024:2048], xs[s][:, 1024:2048],
                        xs[s][:, 1024:2048], op=OP.mult)
                else:
                    nc.gpsimd.tensor_tensor(
                        xsq[:, 1024:2048], xs[s][:, 1024:2048],
                        xs[s][:, 1024:2048], op=OP.mult)
                tr = lpool.tile([128, 2048], f16, tag=f"tr{s}", name=f"tr{s}")
                ns = lpool.tile([128, 64], f32, tag=f"nsq{s}", name=f"nsq{s}")
                w, src_ap, off = 16, xsq[:], 0
                while w >= 1:
                    i0v = src_ap.rearrange("p (f d) -> p f d", d=2 * w)
                    if w == 1:
                        ov = ns[:].unsqueeze(2)
                    else:
                        ov = tr[:, off:off + 64 * w].rearrange(
                            "p (f d) -> p f d", d=w)
                    nc.vector.tensor_tensor(ov, i0v[:, :, 0:w],
                                            i0v[:, :, w:2 * w], op=OP.add)
                    if w > 1:
                        src_ap = tr[:, off:off + 64 * w]
                        off += 64 * w
                    w //= 2
                # nrm = sqrt(nsq): bit seed + 1 Newton iter (Pool), then tanh
                t1 = lpool.tile([128, 64], i32, tag=f"t1{s}", name=f"t1{s}")
                nc.gpsimd.tensor_scalar(t1[:], ns[:].bitcast(i32), 1, None,
                                        op0=OP.logical_shift_right)
                r0 = lpool.tile([128, 64], i32, tag=f"r0{s}", name=f"r0{s}")
                nc.gpsimd.tensor_tensor(
                    r0[:], magic[:].broadcast_to((128, 64)), t1[:],
                    op=OP.subtract)
                u = lpool.tile([128, 64], f32, tag=f"u{s}", name=f"u{s}")
                nc.gpsimd.tensor_tensor(u[:], r0[:].bitcast(f32),
                                        r0[:].bitcast(f32), op=OP.mult)
                v = lpool.tile([128, 64], f32, tag=f"v{s}", name=f"v{s}")
                nc.gpsimd.scalar_tensor_tensor(
                    v[:], ns[:], 0.5, u[:], op0=OP.mult, op1=OP.mult)
                w_ = lpool.tile([128, 64], f32, tag=f"w{s}", name=f"w{s}")
                nc.gpsimd.tensor_scalar(w_[:], v[:], -1.0, 1.5, op0=OP.mult,
                                        op1=OP.add)
                rr = lpool.tile([128, 64], f32, tag=f"rr{s}", name=f"rr{s}")
                nc.gpsimd.tensor_tensor(rr[:], r0[:].bitcast(f32), w_[:],
                                        op=OP.mult)
                nr = lpool.tile([128, 64], f32, tag=f"nrm{s}", name=f"nrm{s}")
                nc.gpsimd.tensor_tensor(nr[:], ns[:], rr[:], op=OP.mult)
                nc.scalar.activation(a_sb[s][:], nr[:], AF.Tanh,
                                     scale=float(C1 ** t))

            def emit_mid(s, t):
                """synT matmuls + copy -> synT_sb[s]."""
                for nh in (0, 1):
                    p0 = nh * 64 + s * 32
                    for mh in (0, 1):
                        nc.tensor.matmul(
                            synT_ps[s][p0:p0 + 32, :],
                            a_sb[s][:, mh * 32:(mh + 1) * 32],
                            wt[:, (mh * 2 + nh) * 128:(mh * 2 + nh + 1) * 128],
                            start=(mh == 0), stop=(mh == 1),
                            tile_position=(0, p0),
                            skip_group_check=True)
                ssb = lpool.tile([128, 128], f16, tag=f"synTs{s}", name=f"synTs{s}")
                nc.scalar.copy(ssb[:], synT_ps[s])
                synT_sb[s] = ssb

            def emit_chunks(s, t):
                """z/gelu both halves, one transpose, then h+V and update."""
                for g in (0, 1):
                    zT = zpp.tile([128, 512], f32, tag="z", name="zT")
                    for k in range(4):
                        O = 8 * g + 4 * s + k
                        nc.tensor.matmul(
                            zT[:, k * 128:(k + 1) * 128],
                            W1d[:, O * 128:(O + 1) * 128], synT_sb[s][:],
                            start=(k == 0), stop=(k == 3),
                            skip_group_check=True)
                    nc.scalar.activation(
                        gT[s][:, g * 512:(g + 1) * 512], zT[:],
                        AF.Gelu, bias=b1v[:, 0:1])
                    if g == 0:
                        sp = xs0p[s][:]
                        vstart = (t == 0)
                        vstop = (t == STEPS - 1)
                    else:
                        spt = spp.tile([128, 1024], f32, tag="sp", name="sp")
                        sp = spt[:]
                        vstart, vstop = True, False
                    for bk in (0, 1):
                        nc.tensor.matmul(
                            sp[:, bk * 512:(bk + 1) * 512],
                            identS[:, t * 128:(t + 1) * 128],
                            V_sb[s][:, g * 1024 + bk * 512:
                                    g * 1024 + (bk + 1) * 512],
                            start=vstart, stop=False,
                            skip_group_check=True)
                    for k in range(4):
                        nc.tensor.matmul(
                            sp[:, k * 256:(k + 1) * 256],
                            gT[s][:, (g * 4 + k) * 128:(g * 4 + k + 1) * 128],
                            bdcs[:, t * 256:(t + 1) * 256],
                            start=False,
                            stop=(vstop and k % 2 == 1) if g == 0
                            else (k % 2 == 1),
                            skip_group_check=True)
                    if g == 1:
                        nc.vector.tensor_tensor(
                            xs[s][:, 1024:2048], xs[s][:, 1024:2048],
                            sp[:], op=OP.add)

            # --- skewed software pipeline: s1 offset half a step ---
            emit_norm(0, 0)
            emit_mid(0, 0)
            emit_norm(1, 0)
            emit_mid(1, 0)
            for t in range(STEPS):
                emit_chunks(0, t)
                if t > 0:
                    emit_norm(1, t)
                    emit_mid(1, t)
                emit_chunks(1, t)
                if t < STEPS - 1:
                    emit_norm(0, t + 1)
                    emit_mid(0, t + 1)

            # drain PSUM-resident halves into xs for output
            for s in (0, 1):
                nc.scalar.copy(xs[s][:, 0:1024], xs0p[s][:])

            # ---------- Phase C: output ----------
            nc.sync.dma_start(xout[:, 0:2048], xs[0][:])
            nc.sync.dma_start(xout[:, 2048:4096], xs[1][:])
    nc.finalize()
    return nc


def _get_nc(n_cores):
    if n_cores not in _cache:
        _cache[n_cores] = build_nc(n_cores)
    return _cache[n_cores]


def kernel(u, features, bias, w_in, b_in, sig_w1, sig_b1, sig_w2, sig_b2):
    from concourse.bass_utils import run_bass_kernel_spmd

    u = np.asarray(u, np.float32)
    args = [np.asarray(a, np.float32) for a in
            (features, bias, w_in, b_in, sig_w1, sig_b1, sig_w2, sig_b2)]
    wt, winTc, baddSm, W1d, b1v, bdcs, identS = _host_prep(*args)

    nc = _get_nc(NCORES)

    in_maps = []
    for c in range(NCORES):
        u_shard = np.ascontiguousarray(
            u[c * BS:(c + 1) * BS].reshape(R, UIN)).astype(np.float16)
        in_maps.append({
            "u_t": u_shard, "wt": wt, "winTc": winTc, "baddSm": baddSm,
            "W1d": W1d, "b1v": b1v, "bdcs": bdcs, "identS": identS,
        })
    res = run_bass_kernel_spmd(nc, in_maps, list(range(NCORES)))

    scale = np.float32(C1 ** STEPS)
    out = np.empty((B, N, D), np.float32)
    for c in range(NCORES):
        xo = res.results[c]["xout"].astype(np.float32) * scale  # [128, 4096]
        # xo[nlo, s*2048 + h*1024 + fcs*32 + d]; b = s*32+fcs, n = h*128+nlo
        v = xo.reshape(128, 2, 2, 32, 32)            # [nlo, s, h, fcs, d]
        out[c * BS:(c + 1) * BS] = (
            v.transpose(1, 3, 2, 0, 4).reshape(BS, N, D))
    return out


# revision 38
# speedup vs baseline: 1.1866x; 1.1866x over previous
"""NeuroPlasticLite Trainium2 kernel (8-core data-parallel over batch).

Layouts (per core, batch shard BS=64, rows r = (b, n), 16384 rows):
  R-layout: SBUF [128 part = nlo, free = fc] with fc = nhi*64 + b,
            n = nhi*128 + nlo.  x free = fc*32 + d.
  T-layout (gT/zT): [128 part = (a, j) = a*16+j, free = O*128 + nlo],
            block O covers fc = 8*O + a.

Scaled basis: xs_t = c1^{-t} x_t, so each step is a pure accumulation
  xs += c1^{-(t+1)} * (DT*h + V)
with the c1 scale folded into per-step consts (identS / bdcs).  Norm uses
act Sqrt(scale=c1^{2t} * nsq + eps).  Final x = c1^20 * xs (host side).

Per step:
  xsq = xs*xs (DVE fp16 2x), nsq = reduce_32 (DVE), nrm = sqrt (Act),
  a = tanh (Act fp16), synT = a @ W blocks (PE, transposed out),
  zT = W1d-expand @ synT (PE), g = gelu(zT + b1) (Act, one op per half),
  s = identS@V + gT @ bdcs (PE, PSUM), xs += s (DVE/Pool TT-add).
"""

import os
from contextlib import ExitStack

import numpy as np

N, D, KF, KN = 256, 32, 16, 50
GAMMA, LAM_A, DT, STEPS = 0.1, 0.95, 0.05, 20
B, UIN = 512, 128
NCORES = 8
BS = B // NCORES          # 64 batch rows per core
R = BS * N                # 16384 rows per core
C1 = 1.0 - DT * GAMMA     # 0.995
EPS = 1e-12

_cache = {}


def _host_prep(features, bias, w_in, b_in, sig_w1, sig_b1, sig_w2, sig_b2):
    """All tiny, replicated tensors (fp16 on-device consts)."""
    f16 = np.float16
    f = features / np.linalg.norm(features, axis=1, keepdims=True)
    sim = f @ f.T                                   # [N, N]
    idx = np.argsort(-sim, axis=1, kind="stable")[:, :KN]        # [N, KN]
    vals = np.take_along_axis(sim, idx, axis=1)                  # [N, KN]
    W = np.zeros((N, N), np.float32)                             # W[m, n]
    np.add.at(W, (idx, np.arange(N)[:, None]), vals)

    # synT-mm rhs blocks: wt[:, (mh*2+nh)*128 + nlo] [mlo, nlo]
    wt = np.concatenate(
        [W[mh * 128:(mh + 1) * 128, nh * 128:(nh + 1) * 128]
         for mh in (0, 1) for nh in (0, 1)], axis=1,
    ).astype(f16)                                                # [128, 512]

    winTc = (DT * w_in.T).astype(f16)                            # [128, 32]
    # baddSm[p=nlo, nhi*32+d] = DT*(bias[n,d] + b_in[d] + sig_b2[d])
    badd = DT * (bias + b_in[None, :] + sig_b2[None, :])         # [256, 32]
    baddSm = np.concatenate([badd[0:128, :], badd[128:256, :]],
                            axis=1).astype(f16)                  # [128, 64]

    # zT-mm lhsT: W1d[fc, O*128 + a*16+j] = (fc == 8*O+a) * w1[j]
    W1d = np.zeros((128, 16 * 128), np.float32)
    w1 = sig_w1[:, 0].astype(np.float32)                         # [16]
    for O in range(16):
        for a in range(8):
            fc = 8 * O + a
            W1d[fc, O * 128 + a * 16:O * 128 + (a + 1) * 16] = w1
    W1d = W1d.astype(f16)                                        # [128, 2048]

    # gelu bias per partition (a,j): b1v[a*16+j] = sig_b1[j]
    b1v = np.tile(sig_b1.astype(np.float32), 8)[:, None]         # [128, 1] f32

    # bd blocks with per-step scale: bdcs[:, t*256 + a'*32+d]
    #   [a*16+j, a'*32+d] = (a==a') * c1^{-(t+1)} * DT * w2[d, j]
    bd0 = np.zeros((128, 256), np.float32)
    for a in range(8):
        bd0[a * 16:(a + 1) * 16, a * 32:(a + 1) * 32] = DT * sig_w2.T
    bdcs = np.concatenate(
        [bd0 * (C1 ** -(t + 1)) for t in range(STEPS)], axis=1
    ).astype(f16)                                                # [128, 5120]

    # identS[:, t*128:(t+1)*128] = c1^{-(t+1)} * I
    ident = np.eye(128, dtype=np.float32)
    identS = np.concatenate(
        [ident * (C1 ** -(t + 1)) for t in range(STEPS)], axis=1
    ).astype(f16)                                                # [128, 2560]

    return wt, winTc, baddSm, W1d, b1v.astype(np.float32), bdcs, identS


def build_nc(n_cores):
    import concourse.bacc as bacc
    import concourse.tile as tile
    from concourse import mybir

    f32 = mybir.dt.float32
    i32 = mybir.dt.int32
    f16 = mybir.dt.float16
    AF = mybir.ActivationFunctionType
    OP = mybir.AluOpType
    AX = mybir.AxisListType

    nc = bacc.Bacc("TRN2", target_bir_lowering=False, debug=False,
                   num_devices=n_cores)
    u_t = nc.declare_dram_parameter("u_t", [R, UIN], f16, isOutput=False)
    wt_d = nc.declare_dram_parameter("wt", [128, 512], f16, isOutput=False)
    winT_d = nc.declare_dram_parameter("winTc", [128, 32], f16, isOutput=False)
    badd_d = nc.declare_dram_parameter("baddSm", [128, 64], f16, isOutput=False)
    w1d_d = nc.declare_dram_parameter("W1d", [128, 2048], f16, isOutput=False)
    b1v_d = nc.declare_dram_parameter("b1v", [128, 1], f32, isOutput=False)
    bdcs_d = nc.declare_dram_parameter("bdcs", [128, 256 * STEPS], f16,
                                       isOutput=False)
    idS_d = nc.declare_dram_parameter("identS", [128, 128 * STEPS], f16,
                                      isOutput=False)
    xout = nc.declare_dram_parameter("xout", [128, 4096], f16, isOutput=True)

    with tile.TileContext(nc) as tc:
        with ExitStack() as ctx:
            cpool = ctx.enter_context(tc.tile_pool(name="consts", bufs=1))
            wt = cpool.tile([128, 512], f16)
            nc.sync.dma_start(wt[:], wt_d[:])
            winTc = cpool.tile([128, 32], f16)
            nc.sync.dma_start(winTc[:], winT_d[:])
            baddSm = cpool.tile([128, 64], f16)
            nc.sync.dma_start(baddSm[:], badd_d[:])
            W1d = cpool.tile([128, 2048], f16)
            nc.sync.dma_start(W1d[:], w1d_d[:])
            b1v = cpool.tile([128, 1], f32)
            nc.sync.dma_start(b1v[:], b1v_d[:])
            bdcs = cpool.tile([128, 256 * STEPS], f16)
            nc.sync.dma_start(bdcs[:], bdcs_d[:])
            identS = cpool.tile([128, 128 * STEPS], f16)
            nc.sync.dma_start(identS[:], idS_d[:])
            magic = cpool.tile([128, 1], mybir.dt.int32)
            nc.vector.memset(magic[:], 0x5F3759DF)

            spool = ctx.enter_context(tc.tile_pool(name="state", bufs=1))
            xs = [spool.tile([128, 2048], f16, name=f"xs{i}") for i in range(2)]
            V_sb = [spool.tile([128, 2048], f16, name=f"V{i}") for i in range(2)]
            gT = [spool.tile([128, 1024], f16, name=f"gT{i}") for i in range(2)]
            gR = [spool.tile([128, 1024], f16, name=f"gR{i}") for i in range(2)]
            uT = spool.tile([128, R], f16)

            nc.vector.memset(xs[0][:], 0.0)
            nc.vector.memset(xs[1][:], 0.0)

            # ---------- Phase B: 20 steps ----------
            lpool = ctx.enter_context(tc.tile_pool(name="loop", bufs=2))
            synp = ctx.enter_context(
                tc.tile_pool(name="synp", bufs=1, space="PSUM"))
            zpp = ctx.enter_context(
                tc.tile_pool(name="zp", bufs=1, space="PSUM"))
            spp = ctx.enter_context(
                tc.tile_pool(name="sp", bufs=1, space="PSUM"))
            xpp = ctx.enter_context(
                tc.tile_pool(name="xp", bufs=1, space="PSUM"))

            # pre-zero synT psum tiles (unwritten partitions must be 0)
            synT_all = synp.tile([128, 256], f32, tag="synT", name="synT")
            synT_ps = [synT_all[:, 0:128], synT_all[:, 128:256]]
            nc.vector.memset(synT_all[:], 0.0)
            # PSUM-resident xs accumulators for the g=0 (nhi=0) half
            xs0p = [xpp.tile([128, 1024], f32, tag=f"xs0p{s}", name=f"xs0p{s}")
                    for s in (0, 1)]

            a_sb = [None, None]
            synT_sb = [None, None]

            def emit_norm(s, t):
                """nsq/sqrt/tanh -> a_sb[s] for step t (reads state of t)."""
                a_sb[s] = lpool.tile([128, 64], f16, tag=f"a{s}", name=f"a{s}")
                if t == 0:
                    nc.vector.memset(a_sb[s][:], 0.0)
                    return
                xsq = lpool.tile([128, 2048], f16, tag=f"xsq{s}", name=f"xsq{s}")
                # g0 half from PSUM (Act), g1 half from SBUF
                nc.scalar.activation(xsq[:, 0:1024], xs0p[s][:], AF.Square)
                if s == 0:
                    nc.vector.tensor_tensor(
                        xsq[:, 1024:2048], xs[s][:, 1024:2048],
                        xs[s][:, 1024:2048], op=OP.mult)
                else:
                    nc.gpsimd.tensor_tensor(
                        xsq[:, 1024:2048], xs[s][:, 1024:2048],
                        xs[s][:, 1024:2048], op=OP.mult)
                tr = lpool.tile([128, 2048], f16, tag=f"tr{s}", name=f"tr{s}")
                ns = lpool.tile([128, 64], f32, tag=f"nsq{s}", name=f"nsq{s}")
                w, src_ap, off = 16, xsq[:], 0
                while w >= 1:
                    i0v = src_ap.rearrange("p (f d) -> p f d", d=2 * w)
                    if w == 1:
                        ov = ns[:].unsqueeze(2)
                    else:
                        ov = tr[:, off:off + 64 * w].rearrange(
                            "p (f d) -> p f d", d=w)
                    nc.vector.tensor_tensor(ov, i0v[:, :, 0:w],
                                            i0v[:, :, w:2 * w], op=OP.add)
                    if w > 1:
                        src_ap = tr[:, off:off + 64 * w]
                        off += 64 * w
                    w //= 2
                # nrm = sqrt(nsq): bit seed + 1 Newton iter (Pool), then tanh
                t1 = lpool.tile([128, 64], i32, tag=f"t1{s}", name=f"t1{s}")
                nc.gpsimd.tensor_scalar(t1[:], ns[:].bitcast(i32), 1, None,
                                        op0=OP.logical_shift_right)
                r0 = lpool.tile([128, 64], i32, tag=f"r0{s}", name=f"r0{s}")
                nc.gpsimd.tensor_tensor(
                    r0[:], magic[:].broadcast_to((128, 64)), t1[:],
                    op=OP.subtract)
                u = lpool.tile([128, 64], f32, tag=f"u{s}", name=f"u{s}")
                nc.gpsimd.tensor_tensor(u[:], r0[:].bitcast(f32),
                                        r0[:].bitcast(f32), op=OP.mult)
                v = lpool.tile([128, 64], f32, tag=f"v{s}", name=f"v{s}")
                nc.gpsimd.scalar_tensor_tensor(
                    v[:], ns[:], 0.5, u[:], op0=OP.mult, op1=OP.mult)
                w_ = lpool.tile([128, 64], f32, tag=f"w{s}", name=f"w{s}")
                nc.gpsimd.tensor_scalar(w_[:], v[:], -1.0, 1.5, op0=OP.mult,
                                        op1=OP.add)
                rr = lpool.tile([128, 64], f32, tag=f"rr{s}", name=f"rr{s}")
                nc.gpsimd.tensor_tensor(rr[:], r0[:].bitcast(f32), w_[:],
                                        op=OP.mult)
                nr = lpool.tile([128, 64], f32, tag=f"nrm{s}", name=f"nrm{s}")
                nc.gpsimd.tensor_tensor(nr[:], ns[:], rr[:], op=OP.mult)
                nc.scalar.activation(a_sb[s][:], nr[:], AF.Tanh,
                                     scale=float(C1 ** t))

            def emit_mid(s, t):
                """synT matmuls + copy -> synT_sb[s]."""
                for nh in (0, 1):
                    p0 = nh * 64 + s * 32
                    for mh in (0, 1):
                        nc.tensor.matmul(
                            synT_ps[s][p0:p0 + 32, :],
                            a_sb[s][:, mh * 32:(mh + 1) * 32],
                            wt[:, (mh * 2 + nh) * 128:(mh * 2 + nh + 1) * 128],
                            start=(mh == 0), stop=(mh == 1),
                            tile_position=(0, p0),
                            skip_group_check=True)
                ssb = lpool.tile([128, 128], f16, tag=f"synTs{s}", name=f"synTs{s}")
                nc.scalar.copy(ssb[:], synT_ps[s])
                synT_sb[s] = ssb

            def emit_chunks(s, t):
                """z/gelu both halves, one transpose, then h+V and update."""
                for g in (0, 1):
                    zT = zpp.tile([128, 512], f32, tag="z", name="zT")
                    for k in range(4):
                        O = 8 * g + 4 * s + k
                        nc.tensor.matmul(
                            zT[:, k * 128:(k + 1) * 128],
                            W1d[:, O * 128:(O + 1) * 128], synT_sb[s][:],
                            start=(k == 0), stop=(k == 3),
                            skip_group_check=True)
                    nc.scalar.activation(
                        gT[s][:, g * 512:(g + 1) * 512], zT[:],
                        AF.Gelu, bias=b1v[:, 0:1])
                    if g == 0:
                        sp = xs0p[s][:]
                        vstart = (t == 0)
                        vstop = (t == STEPS - 1)
                    else:
                        spt = spp.tile([128, 1024], f32, tag="sp", name="sp")
                        sp = spt[:]
                        vstart, vstop = True, False
                    for bk in (0, 1):
                        nc.tensor.matmul(
                            sp[:, bk * 512:(bk + 1) * 512],
                            identS[:, t * 128:(t + 1) * 128],
                            V_sb[s][:, g * 1024 + bk * 512:
                                    g * 1024 + (bk + 1) * 512],
                            start=vstart, stop=False,
                            skip_group_check=True)
                    for k in range(4):
                        nc.tensor.matmul(
                            sp[:, k * 256:(k + 1) * 256],
                            gT[s][:, (g * 4 + k) * 128:(g * 4 + k + 1) * 128],
                            bdcs[:, t * 256:(t + 1) * 256],
                            start=False,
                            stop=(vstop and k % 2 == 1) if g == 0
                            else (k % 2 == 1),
                            skip_group_check=True)
                    if g == 1:
                        nc.vector.tensor_tensor(
                            xs[s][:, 1024:2048], xs[s][:, 1024:2048],
                            sp[:], op=OP.add)

            # prologue (independent of u): overlap with Phase A DMA
            emit_norm(0, 0)
            emit_mid(0, 0)
            emit_norm(1, 0)
            emit_mid(1, 0)

            # ---------- Phase A: transposed u load + u_proj -> V ----------
            if True:
                for ch in range(4):
                    nc.sync.dma_start_transpose(
                        uT[:, ch * 4096:(ch + 1) * 4096],
                        u_t[ch * 4096:(ch + 1) * 4096, :])
                for q in (0, 1, 4, 5, 2, 3, 6, 7):  # stream-0 banks first
                    vp = spp.tile([128, 512], f32, tag="sp", name=f"vp{q}")
                    for k in range(16):
                        fc = 16 * q + k
                        nhi, b = fc // 64, fc % 64
                        r0 = b * 256 + nhi * 128
                        nc.tensor.matmul(
                            vp[:, k * 32:(k + 1) * 32],
                            uT[:, r0:r0 + 128], winTc[:],
                            start=(k % 16 == 0), stop=(k % 16 == 15),
                            skip_group_check=True)
                    nhi = (16 * q) // 64
                    bsl = baddSm[:, nhi * 32:(nhi + 1) * 32]
                    brd = bsl.unsqueeze(1).broadcast_to((128, 16, 32))
                    st, g, sub = (q // 2) % 2, q // 4, (q % 2) * 512
                    nc.vector.tensor_tensor(
                        V_sb[st][:, g * 1024 + sub:g * 1024 + sub + 512]
                        .rearrange("p (s d) -> p s d", d=32),
                        vp[:].rearrange("p (s d) -> p s d", d=32),
                        brd, op=OP.add)


            for t in range(STEPS):
                emit_chunks(0, t)
                if t > 0:
                    emit_norm(1, t)
                    emit_mid(1, t)
                if t < STEPS - 1:
                    emit_norm(0, t + 1)
                emit_chunks(1, t)
                if t < STEPS - 1:
                    emit_mid(0, t + 1)

            # drain PSUM-resident halves into xs for output
            for s in (0, 1):
                nc.scalar.copy(xs[s][:, 0:1024], xs0p[s][:])

            # ---------- Phase C: output ----------
            nc.sync.dma_start(xout[:, 0:2048], xs[0][:])
            nc.sync.dma_start(xout[:, 2048:4096], xs[1][:])
    nc.finalize()
    return nc


def _get_nc(n_cores):
    if n_cores not in _cache:
        _cache[n_cores] = build_nc(n_cores)
    return _cache[n_cores]


def kernel(u, features, bias, w_in, b_in, sig_w1, sig_b1, sig_w2, sig_b2):
    from concourse.bass_utils import run_bass_kernel_spmd

    u = np.asarray(u, np.float32)
    args = [np.asarray(a, np.float32) for a in
            (features, bias, w_in, b_in, sig_w1, sig_b1, sig_w2, sig_b2)]
    wt, winTc, baddSm, W1d, b1v, bdcs, identS = _host_prep(*args)

    nc = _get_nc(NCORES)

    in_maps = []
    for c in range(NCORES):
        u_shard = np.ascontiguousarray(
            u[c * BS:(c + 1) * BS].reshape(R, UIN)).astype(np.float16)
        in_maps.append({
            "u_t": u_shard, "wt": wt, "winTc": winTc, "baddSm": baddSm,
            "W1d": W1d, "b1v": b1v, "bdcs": bdcs, "identS": identS,
        })
    res = run_bass_kernel_spmd(nc, in_maps, list(range(NCORES)))

    scale = np.float32(C1 ** STEPS)
    out = np.empty((B, N, D), np.float32)
    for c in range(NCORES):
        xo = res.results[c]["xout"].astype(np.float32) * scale  # [128, 4096]
        # xo[nlo, s*2048 + h*1024 + fcs*32 + d]; b = s*32+fcs, n = h*128+nlo
        v = xo.reshape(128, 2, 2, 32, 32)            # [nlo, s, h, fcs, d]
        out[c * BS:(c + 1) * BS] = (
            v.transpose(1, 3, 2, 0, 4).reshape(BS, N, D))
    return out


# revision 39
# speedup vs baseline: 1.1870x; 1.0004x over previous
"""NeuroPlasticLite Trainium2 kernel (8-core data-parallel over batch).

Layouts (per core, batch shard BS=64, rows r = (b, n), 16384 rows):
  R-layout: SBUF [128 part = nlo, free = fc] with fc = nhi*64 + b,
            n = nhi*128 + nlo.  x free = fc*32 + d.
  T-layout (gT/zT): [128 part = (a, j) = a*16+j, free = O*128 + nlo],
            block O covers fc = 8*O + a.

Scaled basis: xs_t = c1^{-t} x_t, so each step is a pure accumulation
  xs += c1^{-(t+1)} * (DT*h + V)
with the c1 scale folded into per-step consts (identS / bdcs).  Norm uses
act Sqrt(scale=c1^{2t} * nsq + eps).  Final x = c1^20 * xs (host side).

Per step:
  xsq = xs*xs (DVE fp16 2x), nsq = reduce_32 (DVE), nrm = sqrt (Act),
  a = tanh (Act fp16), synT = a @ W blocks (PE, transposed out),
  zT = W1d-expand @ synT (PE), g = gelu(zT + b1) (Act, one op per half),
  s = identS@V + gT @ bdcs (PE, PSUM), xs += s (DVE/Pool TT-add).
"""

import os
from contextlib import ExitStack

import numpy as np

N, D, KF, KN = 256, 32, 16, 50
GAMMA, LAM_A, DT, STEPS = 0.1, 0.95, 0.05, 20
B, UIN = 512, 128
NCORES = 8
BS = B // NCORES          # 64 batch rows per core
R = BS * N                # 16384 rows per core
C1 = 1.0 - DT * GAMMA     # 0.995
EPS = 1e-12

_cache = {}


def _host_prep(features, bias, w_in, b_in, sig_w1, sig_b1, sig_w2, sig_b2):
    """All tiny, replicated tensors (fp16 on-device consts)."""
    f16 = np.float16
    f = features / np.linalg.norm(features, axis=1, keepdims=True)
    sim = f @ f.T                                   # [N, N]
    idx = np.argsort(-sim, axis=1, kind="stable")[:, :KN]        # [N, KN]
    vals = np.take_along_axis(sim, idx, axis=1)                  # [N, KN]
    W = np.zeros((N, N), np.float32)                             # W[m, n]
    np.add.at(W, (idx, np.arange(N)[:, None]), vals)

    # synT-mm rhs blocks: wt[:, (mh*2+nh)*128 + nlo] [mlo, nlo]
    wt = np.concatenate(
        [W[mh * 128:(mh + 1) * 128, nh * 128:(nh + 1) * 128]
         for mh in (0, 1) for nh in (0, 1)], axis=1,
    ).astype(f16)                                                # [128, 512]

    winTc = (DT * w_in.T).astype(f16)                            # [128, 32]
    # baddSm[p=nlo, nhi*32+d] = DT*(bias[n,d] + b_in[d] + sig_b2[d])
    badd = DT * (bias + b_in[None, :] + sig_b2[None, :])         # [256, 32]
    baddSm = np.concatenate([badd[0:128, :], badd[128:256, :]],
                            axis=1).astype(f16)                  # [128, 64]

    # zT-mm lhsT: W1d[fc, O*128 + a*16+j] = (fc == 8*O+a) * w1[j]
    W1d = np.zeros((128, 16 * 128), np.float32)
    w1 = sig_w1[:, 0].astype(np.float32)                         # [16]
    for O in range(16):
        for a in range(8):
            fc = 8 * O + a
            W1d[fc, O * 128 + a * 16:O * 128 + (a + 1) * 16] = w1
    W1d = W1d.astype(f16)                                        # [128, 2048]

    # gelu bias per partition (a,j): b1v[a*16+j] = sig_b1[j]
    b1v = np.tile(sig_b1.astype(np.float32), 8)[:, None]         # [128, 1] f32

    # bd blocks with per-step scale: bdcs[:, t*256 + a'*32+d]
    #   [a*16+j, a'*32+d] = (a==a') * c1^{-(t+1)} * DT * w2[d, j]
    bd0 = np.zeros((128, 256), np.float32)
    for a in range(8):
        bd0[a * 16:(a + 1) * 16, a * 32:(a + 1) * 32] = DT * sig_w2.T
    bdcs = np.concatenate(
        [bd0 * (C1 ** -(t + 1)) for t in range(STEPS)], axis=1
    ).astype(f16)                                                # [128, 5120]

    # identS[:, t*128:(t+1)*128] = c1^{-(t+1)} * I
    ident = np.eye(128, dtype=np.float32)
    identS = np.concatenate(
        [ident * (C1 ** -(t + 1)) for t in range(STEPS)], axis=1
    ).astype(f16)                                                # [128, 2560]

    return wt, winTc, baddSm, W1d, b1v.astype(np.float32), bdcs, identS


def build_nc(n_cores):
    import concourse.bacc as bacc
    import concourse.tile as tile
    from concourse import mybir

    f32 = mybir.dt.float32
    i32 = mybir.dt.int32
    f16 = mybir.dt.float16
    AF = mybir.ActivationFunctionType
    OP = mybir.AluOpType
    AX = mybir.AxisListType

    nc = bacc.Bacc("TRN2", target_bir_lowering=False, debug=False,
                   num_devices=n_cores)
    u_t = nc.declare_dram_parameter("u_t", [R, UIN], f16, isOutput=False)
    wt_d = nc.declare_dram_parameter("wt", [128, 512], f16, isOutput=False)
    winT_d = nc.declare_dram_parameter("winTc", [128, 32], f16, isOutput=False)
    badd_d = nc.declare_dram_parameter("baddSm", [128, 64], f16, isOutput=False)
    w1d_d = nc.declare_dram_parameter("W1d", [128, 2048], f16, isOutput=False)
    b1v_d = nc.declare_dram_parameter("b1v", [128, 1], f32, isOutput=False)
    bdcs_d = nc.declare_dram_parameter("bdcs", [128, 256 * STEPS], f16,
                                       isOutput=False)
    idS_d = nc.declare_dram_parameter("identS", [128, 128 * STEPS], f16,
                                      isOutput=False)
    xout = nc.declare_dram_parameter("xout", [128, 4096], f16, isOutput=True)

    with tile.TileContext(nc) as tc:
        with ExitStack() as ctx:
            cpool = ctx.enter_context(tc.tile_pool(name="consts", bufs=1))
            wt = cpool.tile([128, 512], f16)
            nc.sync.dma_start(wt[:], wt_d[:])
            winTc = cpool.tile([128, 32], f16)
            nc.sync.dma_start(winTc[:], winT_d[:])
            baddSm = cpool.tile([128, 64], f16)
            nc.sync.dma_start(baddSm[:], badd_d[:])
            W1d = cpool.tile([128, 2048], f16)
            nc.sync.dma_start(W1d[:], w1d_d[:])
            b1v = cpool.tile([128, 1], f32)
            nc.sync.dma_start(b1v[:], b1v_d[:])
            bdcs = cpool.tile([128, 256 * STEPS], f16)
            nc.sync.dma_start(bdcs[:], bdcs_d[:])
            identS = cpool.tile([128, 128 * STEPS], f16)
            nc.sync.dma_start(identS[:], idS_d[:])
            magic = cpool.tile([128, 1], mybir.dt.int32)
            nc.vector.memset(magic[:], 0x5F3759DF)

            spool = ctx.enter_context(tc.tile_pool(name="state", bufs=1))
            xs = [spool.tile([128, 2048], f16, name=f"xs{i}") for i in range(2)]
            V_sb = [spool.tile([128, 2048], f16, name=f"V{i}") for i in range(2)]
            gT = [spool.tile([128, 1024], f16, name=f"gT{i}") for i in range(2)]
            gR = [spool.tile([128, 1024], f16, name=f"gR{i}") for i in range(2)]
            uT = spool.tile([128, R], f16)

            nc.vector.memset(xs[0][:], 0.0)
            nc.vector.memset(xs[1][:], 0.0)

            # ---------- Phase B: 20 steps ----------
            lpool = ctx.enter_context(tc.tile_pool(name="loop", bufs=2))
            synp = ctx.enter_context(
                tc.tile_pool(name="synp", bufs=1, space="PSUM"))
            zpp = ctx.enter_context(
                tc.tile_pool(name="zp", bufs=1, space="PSUM"))
            spp = ctx.enter_context(
                tc.tile_pool(name="sp", bufs=1, space="PSUM"))
            xpp = ctx.enter_context(
                tc.tile_pool(name="xp", bufs=1, space="PSUM"))

            # pre-zero synT psum tiles (unwritten partitions must be 0)
            synT_all = synp.tile([128, 256], f32, tag="synT", name="synT")
            synT_ps = [synT_all[:, 0:128], synT_all[:, 128:256]]
            nc.vector.memset(synT_all[:], 0.0)
            # PSUM-resident xs accumulators for the g=0 (nhi=0) half
            xs0p = [xpp.tile([128, 1024], f32, tag=f"xs0p{s}", name=f"xs0p{s}")
                    for s in (0, 1)]

            a_sb = [None, None]
            synT_sb = [None, None]

            def emit_norm(s, t):
                """nsq/sqrt/tanh -> a_sb[s] for step t (reads state of t)."""
                a_sb[s] = lpool.tile([128, 64], f16, tag=f"a{s}", name=f"a{s}")
                if t == 0:
                    nc.vector.memset(a_sb[s][:], 0.0)
                    return
                xsq = lpool.tile([128, 2048], f16, tag=f"xsq{s}", name=f"xsq{s}")
                # g0 half from PSUM (Act), g1 half from SBUF
                nc.scalar.activation(xsq[:, 0:1024], xs0p[s][:], AF.Square)
                if s == 0:
                    nc.vector.tensor_tensor(
                        xsq[:, 1024:2048], xs[s][:, 1024:2048],
                        xs[s][:, 1024:2048], op=OP.mult)
                else:
                    nc.gpsimd.tensor_tensor(
                        xsq[:, 1024:2048], xs[s][:, 1024:2048],
                        xs[s][:, 1024:2048], op=OP.mult)
                tr = lpool.tile([128, 2048], f16, tag=f"tr{s}", name=f"tr{s}")
                ns = lpool.tile([128, 64], f32, tag=f"nsq{s}", name=f"nsq{s}")
                w, src_ap, off = 16, xsq[:], 0
                while w >= 1:
                    i0v = src_ap.rearrange("p (f d) -> p f d", d=2 * w)
                    if w == 1:
                        ov = ns[:].unsqueeze(2)
                    else:
                        ov = tr[:, off:off + 64 * w].rearrange(
                            "p (f d) -> p f d", d=w)
                    nc.vector.tensor_tensor(ov, i0v[:, :, 0:w],
                                            i0v[:, :, w:2 * w], op=OP.add)
                    if w > 1:
                        src_ap = tr[:, off:off + 64 * w]
                        off += 64 * w
                    w //= 2
                # nrm = sqrt(nsq): bit seed + 1 Newton iter (Pool), then tanh
                t1 = lpool.tile([128, 64], i32, tag=f"t1{s}", name=f"t1{s}")
                nc.gpsimd.tensor_scalar(t1[:], ns[:].bitcast(i32), 1, None,
                                        op0=OP.logical_shift_right)
                r0 = lpool.tile([128, 64], i32, tag=f"r0{s}", name=f"r0{s}")
                nc.gpsimd.tensor_tensor(
                    r0[:], magic[:].broadcast_to((128, 64)), t1[:],
                    op=OP.subtract)
                u = lpool.tile([128, 64], f32, tag=f"u{s}", name=f"u{s}")
                nc.gpsimd.tensor_tensor(u[:], r0[:].bitcast(f32),
                                        r0[:].bitcast(f32), op=OP.mult)
                v = lpool.tile([128, 64], f32, tag=f"v{s}", name=f"v{s}")
                nc.gpsimd.scalar_tensor_tensor(
                    v[:], ns[:], 0.5, u[:], op0=OP.mult, op1=OP.mult)
                w_ = lpool.tile([128, 64], f32, tag=f"w{s}", name=f"w{s}")
                nc.gpsimd.tensor_scalar(w_[:], v[:], -1.0, 1.5, op0=OP.mult,
                                        op1=OP.add)
                rr = lpool.tile([128, 64], f32, tag=f"rr{s}", name=f"rr{s}")
                nc.gpsimd.tensor_tensor(rr[:], r0[:].bitcast(f32), w_[:],
                                        op=OP.mult)
                nr = lpool.tile([128, 64], f32, tag=f"nrm{s}", name=f"nrm{s}")
                nc.gpsimd.tensor_tensor(nr[:], ns[:], rr[:], op=OP.mult)
                nc.scalar.activation(a_sb[s][:], nr[:], AF.Tanh,
                                     scale=float(C1 ** t))

            def emit_mid(s, t):
                """synT matmuls + copy -> synT_sb[s]."""
                for nh in (0, 1):
                    p0 = nh * 64 + s * 32
                    for mh in (0, 1):
                        nc.tensor.matmul(
                            synT_ps[s][p0:p0 + 32, :],
                            a_sb[s][:, mh * 32:(mh + 1) * 32],
                            wt[:, (mh * 2 + nh) * 128:(mh * 2 + nh + 1) * 128],
                            start=(mh == 0), stop=(mh == 1),
                            tile_position=(0, p0),
                            skip_group_check=True)
                ssb = lpool.tile([128, 128], f16, tag=f"synTs{s}", name=f"synTs{s}")
                nc.scalar.copy(ssb[:], synT_ps[s])
                synT_sb[s] = ssb

            def emit_chunks(s, t):
                """z/gelu both halves, one transpose, then h+V and update."""
                for g in (0, 1):
                    zT = zpp.tile([128, 512], f32, tag="z", name="zT")
                    for k in range(4):
                        O = 8 * g + 4 * s + k
                        nc.tensor.matmul(
                            zT[:, k * 128:(k + 1) * 128],
                            W1d[:, O * 128:(O + 1) * 128], synT_sb[s][:],
                            start=(k == 0), stop=(k == 3),
                            skip_group_check=True)
                    nc.scalar.activation(
                        gT[s][:, g * 512:(g + 1) * 512], zT[:],
                        AF.Gelu, bias=b1v[:, 0:1])
                    if g == 0:
                        sp = xs0p[s][:]
                        vstart = (t == 0)
                        vstop = (t == STEPS - 1)
                    else:
                        spt = spp.tile([128, 1024], f32, tag="sp", name="sp")
                        sp = spt[:]
                        vstart, vstop = True, False
                    for bk in (0, 1):
                        nc.tensor.matmul(
                            sp[:, bk * 512:(bk + 1) * 512],
                            identS[:, t * 128:(t + 1) * 128],
                            V_sb[s][:, g * 1024 + bk * 512:
                                    g * 1024 + (bk + 1) * 512],
                            start=vstart, stop=False,
                            skip_group_check=True)
                    for k in range(4):
                        nc.tensor.matmul(
                            sp[:, k * 256:(k + 1) * 256],
                            gT[s][:, (g * 4 + k) * 128:(g * 4 + k + 1) * 128],
                            bdcs[:, t * 256:(t + 1) * 256],
                            start=False,
                            stop=(vstop and k % 2 == 1) if g == 0
                            else (k % 2 == 1),
                            skip_group_check=True)
                    if g == 1:
                        nc.vector.tensor_tensor(
                            xs[s][:, 1024:2048], xs[s][:, 1024:2048],
                            sp[:], op=OP.add)

            # prologue (independent of u): overlap with Phase A DMA
            emit_norm(0, 0)
            emit_mid(0, 0)
            emit_norm(1, 0)
            emit_mid(1, 0)

            # ---------- Phase A: transposed u load + u_proj -> V ----------
            if True:
                for ch in range(8):
                    nc.sync.dma_start_transpose(
                        uT[:, ch * 2048:(ch + 1) * 2048],
                        u_t[ch * 2048:(ch + 1) * 2048, :])
                for q in (0, 1, 4, 5, 2, 3, 6, 7):  # stream-0 banks first
                    vp = spp.tile([128, 512], f32, tag="sp", name=f"vp{q}")
                    for k in range(16):
                        fc = 16 * q + k
                        nhi, b = fc // 64, fc % 64
                        r0 = b * 256 + nhi * 128
                        nc.tensor.matmul(
                            vp[:, k * 32:(k + 1) * 32],
                            uT[:, r0:r0 + 128], winTc[:],
                            start=(k % 16 == 0), stop=(k % 16 == 15),
                            skip_group_check=True)
                    nhi = (16 * q) // 64
                    bsl = baddSm[:, nhi * 32:(nhi + 1) * 32]
                    brd = bsl.unsqueeze(1).broadcast_to((128, 16, 32))
                    st, g, sub = (q // 2) % 2, q // 4, (q % 2) * 512
                    nc.vector.tensor_tensor(
                        V_sb[st][:, g * 1024 + sub:g * 1024 + sub + 512]
                        .rearrange("p (s d) -> p s d", d=32),
                        vp[:].rearrange("p (s d) -> p s d", d=32),
                        brd, op=OP.add)


            for t in range(STEPS):
                emit_chunks(0, t)
                if t > 0:
                    emit_norm(1, t)
                    emit_mid(1, t)
                if t < STEPS - 1:
                    emit_norm(0, t + 1)
                emit_chunks(1, t)
                if t < STEPS - 1:
                    emit_mid(0, t + 1)

            # drain PSUM-resident halves into xs for output
            for s in (0, 1):
                nc.scalar.copy(xs[s][:, 0:1024], xs0p[s][:])

            # ---------- Phase C: output ----------
            nc.sync.dma_start(xout[:, 0:2048], xs[0][:])
            nc.sync.dma_start(xout[:, 2048:4096], xs[1][:])
    nc.finalize()
    return nc


def _get_nc(n_cores):
    if n_cores not in _cache:
        _cache[n_cores] = build_nc(n_cores)
    return _cache[n_cores]


def kernel(u, features, bias, w_in, b_in, sig_w1, sig_b1, sig_w2, sig_b2):
    from concourse.bass_utils import run_bass_kernel_spmd

    u = np.asarray(u, np.float32)
    args = [np.asarray(a, np.float32) for a in
            (features, bias, w_in, b_in, sig_w1, sig_b1, sig_w2, sig_b2)]
    wt, winTc, baddSm, W1d, b1v, bdcs, identS = _host_prep(*args)

    nc = _get_nc(NCORES)

    in_maps = []
    for c in range(NCORES):
        u_shard = np.ascontiguousarray(
            u[c * BS:(c + 1) * BS].reshape(R, UIN)).astype(np.float16)
        in_maps.append({
            "u_t": u_shard, "wt": wt, "winTc": winTc, "baddSm": baddSm,
            "W1d": W1d, "b1v": b1v, "bdcs": bdcs, "identS": identS,
        })
    res = run_bass_kernel_spmd(nc, in_maps, list(range(NCORES)))

    scale = np.float32(C1 ** STEPS)
    out = np.empty((B, N, D), np.float32)
    for c in range(NCORES):
        xo = res.results[c]["xout"].astype(np.float32) * scale  # [128, 4096]
        # xo[nlo, s*2048 + h*1024 + fcs*32 + d]; b = s*32+fcs, n = h*128+nlo
        v = xo.reshape(128, 2, 2, 32, 32)            # [nlo, s, h, fcs, d]
        out[c * BS:(c + 1) * BS] = (
            v.transpose(1, 3, 2, 0, 4).reshape(BS, N, D))
    return out


# revision 45
# speedup vs baseline: 1.1897x; 1.0023x over previous
"""NeuroPlasticLite Trainium2 kernel (8-core data-parallel over batch).

Layouts (per core, batch shard BS=64, rows r = (b, n), 16384 rows):
  R-layout: SBUF [128 part = nlo, free = fc] with fc = nhi*64 + b,
            n = nhi*128 + nlo.  x free = fc*32 + d.
  T-layout (gT/zT): [128 part = (a, j) = a*16+j, free = O*128 + nlo],
            block O covers fc = 8*O + a.

Scaled basis: xs_t = c1^{-t} x_t, so each step is a pure accumulation
  xs += c1^{-(t+1)} * (DT*h + V)
with the c1 scale folded into per-step consts (identS / bdcs).  Norm uses
act Sqrt(scale=c1^{2t} * nsq + eps).  Final x = c1^20 * xs (host side).

Per step:
  xsq = xs*xs (DVE fp16 2x), nsq = reduce_32 (DVE), nrm = sqrt (Act),
  a = tanh (Act fp16), synT = a @ W blocks (PE, transposed out),
  zT = W1d-expand @ synT (PE), g = gelu(zT + b1) (Act, one op per half),
  s = identS@V + gT @ bdcs (PE, PSUM), xs += s (DVE/Pool TT-add).
"""

import os
from contextlib import ExitStack

import numpy as np

N, D, KF, KN = 256, 32, 16, 50
GAMMA, LAM_A, DT, STEPS = 0.1, 0.95, 0.05, 20
B, UIN = 512, 128
NCORES = 8
BS = B // NCORES          # 64 batch rows per core
R = BS * N                # 16384 rows per core
C1 = 1.0 - DT * GAMMA     # 0.995
EPS = 1e-12

_cache = {}


def _host_prep(features, bias, w_in, b_in, sig_w1, sig_b1, sig_w2, sig_b2):
    """All tiny, replicated tensors (fp16 on-device consts)."""
    f16 = np.float16
    f = features / np.linalg.norm(features, axis=1, keepdims=True)
    sim = f @ f.T                                   # [N, N]
    idx = np.argsort(-sim, axis=1, kind="stable")[:, :KN]        # [N, KN]
    vals = np.take_along_axis(sim, idx, axis=1)                  # [N, KN]
    W = np.zeros((N, N), np.float32)                             # W[m, n]
    np.add.at(W, (idx, np.arange(N)[:, None]), vals)

    # synT-mm rhs blocks: wt[:, (mh*2+nh)*128 + nlo] [mlo, nlo]
    wt = np.concatenate(
        [W[mh * 128:(mh + 1) * 128, nh * 128:(nh + 1) * 128]
         for mh in (0, 1) for nh in (0, 1)], axis=1,
    ).astype(f16)                                                # [128, 512]

    winTc = (DT * w_in.T).astype(f16)                            # [128, 32]
    # baddSm[p=nlo, nhi*32+d] = DT*(bias[n,d] + b_in[d] + sig_b2[d])
    badd = DT * (bias + b_in[None, :] + sig_b2[None, :])         # [256, 32]
    baddSm = np.concatenate([badd[0:128, :], badd[128:256, :]],
                            axis=1).astype(f16)                  # [128, 64]

    # zT-mm lhsT: W1d[fc, O*128 + a*16+j] = (fc == 8*O+a) * w1[j]
    W1d = np.zeros((128, 16 * 128), np.float32)
    w1 = sig_w1[:, 0].astype(np.float32)                         # [16]
    for O in range(16):
        for a in range(8):
            fc = 8 * O + a
            W1d[fc, O * 128 + a * 16:O * 128 + (a + 1) * 16] = w1
    W1d = W1d.astype(f16)                                        # [128, 2048]

    # gelu bias per partition (a,j): b1v[a*16+j] = sig_b1[j]
    b1v = np.tile(sig_b1.astype(np.float32), 8)[:, None]         # [128, 1] f32

    # bd blocks with per-step scale: bdcs[:, t*256 + a'*32+d]
    #   [a*16+j, a'*32+d] = (a==a') * c1^{-(t+1)} * DT * w2[d, j]
    bd0 = np.zeros((128, 256), np.float32)
    for a in range(8):
        bd0[a * 16:(a + 1) * 16, a * 32:(a + 1) * 32] = DT * sig_w2.T
    bdcs = np.concatenate(
        [bd0 * (C1 ** -(t + 1)) for t in range(STEPS)], axis=1
    ).astype(f16)                                                # [128, 5120]

    # identS[:, t*128:(t+1)*128] = c1^{-(t+1)} * I
    ident = np.eye(128, dtype=np.float32)
    identS = np.concatenate(
        [ident * (C1 ** -(t + 1)) for t in range(STEPS)], axis=1
    ).astype(f16)                                                # [128, 2560]

    return wt, winTc, baddSm, W1d, b1v.astype(np.float32), bdcs, identS


def build_nc(n_cores):
    import concourse.bacc as bacc
    import concourse.tile as tile
    from concourse import mybir

    f32 = mybir.dt.float32
    i32 = mybir.dt.int32
    f16 = mybir.dt.float16
    AF = mybir.ActivationFunctionType
    OP = mybir.AluOpType
    AX = mybir.AxisListType

    nc = bacc.Bacc("TRN2", target_bir_lowering=False, debug=False,
                   num_devices=n_cores)
    u_t = nc.declare_dram_parameter("u_t", [R, UIN], f16, isOutput=False)
    wt_d = nc.declare_dram_parameter("wt", [128, 512], f16, isOutput=False)
    winT_d = nc.declare_dram_parameter("winTc", [128, 32], f16, isOutput=False)
    badd_d = nc.declare_dram_parameter("baddSm", [128, 64], f16, isOutput=False)
    w1d_d = nc.declare_dram_parameter("W1d", [128, 2048], f16, isOutput=False)
    b1v_d = nc.declare_dram_parameter("b1v", [128, 1], f32, isOutput=False)
    bdcs_d = nc.declare_dram_parameter("bdcs", [128, 256 * STEPS], f16,
                                       isOutput=False)
    idS_d = nc.declare_dram_parameter("identS", [128, 128 * STEPS], f16,
                                      isOutput=False)
    xout = nc.declare_dram_parameter("xout", [128, 4096], f16, isOutput=True)

    with tile.TileContext(nc) as tc:
        with ExitStack() as ctx:
            cpool = ctx.enter_context(tc.tile_pool(name="consts", bufs=1))
            wt = cpool.tile([128, 512], f16)
            nc.sync.dma_start(wt[:], wt_d[:])
            winTc = cpool.tile([128, 32], f16)
            nc.sync.dma_start(winTc[:], winT_d[:])
            baddSm = cpool.tile([128, 64], f16)
            nc.sync.dma_start(baddSm[:], badd_d[:])
            W1d = cpool.tile([128, 2048], f16)
            nc.sync.dma_start(W1d[:], w1d_d[:])
            b1v = cpool.tile([128, 1], f32)
            nc.sync.dma_start(b1v[:], b1v_d[:])
            bdcs = cpool.tile([128, 256 * STEPS], f16)
            nc.sync.dma_start(bdcs[:], bdcs_d[:])
            identS = cpool.tile([128, 128 * STEPS], f16)
            nc.sync.dma_start(identS[:], idS_d[:])
            magic = cpool.tile([128, 1], mybir.dt.int32)
            nc.vector.memset(magic[:], 0x5F3759DF)

            spool = ctx.enter_context(tc.tile_pool(name="state", bufs=1))
            xs = [spool.tile([128, 2048], f16, name=f"xs{i}") for i in range(2)]
            V_sb = [spool.tile([128, 2048], f16, name=f"V{i}") for i in range(2)]
            gT = [spool.tile([128, 1024], f16, name=f"gT{i}") for i in range(2)]
            gR = [spool.tile([128, 1024], f16, name=f"gR{i}") for i in range(2)]
            uT = spool.tile([128, R], f16)

            nc.vector.memset(xs[0][:], 0.0)
            nc.vector.memset(xs[1][:], 0.0)

            # ---------- Phase B: 20 steps ----------
            lpool = ctx.enter_context(tc.tile_pool(name="loop", bufs=2))
            synp = ctx.enter_context(
                tc.tile_pool(name="synp", bufs=1, space="PSUM"))
            zpp = ctx.enter_context(
                tc.tile_pool(name="zp", bufs=1, space="PSUM"))
            spp = ctx.enter_context(
                tc.tile_pool(name="sp", bufs=1, space="PSUM"))
            xpp = ctx.enter_context(
                tc.tile_pool(name="xp", bufs=1, space="PSUM"))

            # pre-zero synT psum tiles (unwritten partitions must be 0)
            synT_all = synp.tile([128, 256], f32, tag="synT", name="synT")
            synT_ps = [synT_all[:, 0:128], synT_all[:, 128:256]]
            nc.vector.memset(synT_all[:], 0.0)
            # PSUM-resident xs accumulators for the g=0 (nhi=0) half
            xs0p = [xpp.tile([128, 1024], f32, tag=f"xs0p{s}", name=f"xs0p{s}")
                    for s in (0, 1)]

            a_sb = [None, None]
            synT_sb = [None, None]

            def emit_norm(s, t):
                """nsq/sqrt/tanh -> a_sb[s] for step t (reads state of t)."""
                a_sb[s] = lpool.tile([128, 64], f16, tag=f"a{s}", name=f"a{s}")
                if t == 0:
                    nc.vector.memset(a_sb[s][:], 0.0)
                    return
                xsq = lpool.tile([128, 2048], f16, tag=f"xsq{s}", name=f"xsq{s}")
                # g0 half from PSUM (Act), g1 half from SBUF
                nc.scalar.activation(xsq[:, 0:1024], xs0p[s][:], AF.Square)
                if s == 0:
                    nc.vector.tensor_tensor(
                        xsq[:, 1024:2048], xs[s][:, 1024:2048],
                        xs[s][:, 1024:2048], op=OP.mult)
                else:
                    nc.gpsimd.tensor_tensor(
                        xsq[:, 1024:2048], xs[s][:, 1024:2048],
                        xs[s][:, 1024:2048], op=OP.mult)
                tr = lpool.tile([128, 2048], f16, tag=f"tr{s}", name=f"tr{s}")
                ns = lpool.tile([128, 64], f32, tag=f"nsq{s}", name=f"nsq{s}")
                w, src_ap, off = 16, xsq[:], 0
                while w >= 1:
                    i0v = src_ap.rearrange("p (f d) -> p f d", d=2 * w)
                    if w == 1:
                        ov = ns[:].unsqueeze(2)
                    else:
                        ov = tr[:, off:off + 64 * w].rearrange(
                            "p (f d) -> p f d", d=w)
                    nc.vector.tensor_tensor(ov, i0v[:, :, 0:w],
                                            i0v[:, :, w:2 * w], op=OP.add)
                    if w > 1:
                        src_ap = tr[:, off:off + 64 * w]
                        off += 64 * w
                    w //= 2
                # nrm = sqrt(nsq): bit seed + 1 Newton iter (Pool), then tanh
                t1 = lpool.tile([128, 64], i32, tag=f"t1{s}", name=f"t1{s}")
                nc.gpsimd.tensor_scalar(t1[:], ns[:].bitcast(i32), 1, None,
                                        op0=OP.logical_shift_right)
                r0 = lpool.tile([128, 64], i32, tag=f"r0{s}", name=f"r0{s}")
                nc.gpsimd.tensor_tensor(
                    r0[:], magic[:].broadcast_to((128, 64)), t1[:],
                    op=OP.subtract)
                u = lpool.tile([128, 64], f32, tag=f"u{s}", name=f"u{s}")
                nc.gpsimd.tensor_tensor(u[:], r0[:].bitcast(f32),
                                        r0[:].bitcast(f32), op=OP.mult)
                v = lpool.tile([128, 64], f32, tag=f"v{s}", name=f"v{s}")
                nc.gpsimd.scalar_tensor_tensor(
                    v[:], ns[:], 0.5, u[:], op0=OP.mult, op1=OP.mult)
                w_ = lpool.tile([128, 64], f32, tag=f"w{s}", name=f"w{s}")
                nc.gpsimd.tensor_scalar(w_[:], v[:], -1.0, 1.5, op0=OP.mult,
                                        op1=OP.add)
                rr = lpool.tile([128, 64], f32, tag=f"rr{s}", name=f"rr{s}")
                nc.gpsimd.tensor_tensor(rr[:], r0[:].bitcast(f32), w_[:],
                                        op=OP.mult)
                nr = lpool.tile([128, 64], f32, tag=f"nrm{s}", name=f"nrm{s}")
                nc.gpsimd.tensor_tensor(nr[:], ns[:], rr[:], op=OP.mult)
                nc.scalar.activation(a_sb[s][:], nr[:], AF.Tanh,
                                     scale=float(C1 ** t))

            def emit_mid(s, t):
                """synT matmuls + copy -> synT_sb[s]."""
                for nh in (0, 1):
                    p0 = nh * 64 + s * 32
                    for mh in (0, 1):
                        nc.tensor.matmul(
                            synT_ps[s][p0:p0 + 32, :],
                            a_sb[s][:, mh * 32:(mh + 1) * 32],
                            wt[:, (mh * 2 + nh) * 128:(mh * 2 + nh + 1) * 128],
                            start=(mh == 0), stop=(mh == 1),
                            tile_position=(0, p0),
                            skip_group_check=True)
                ssb = lpool.tile([128, 128], f16, tag=f"synTs{s}", name=f"synTs{s}")
                nc.scalar.copy(ssb[:], synT_ps[s])
                synT_sb[s] = ssb

            def emit_chunks(s, t):
                """z/gelu both halves then h+V and update (h skipped at t=0
                since a_0 ~ 0 makes the gelu output exactly 0)."""
                for g in (0, 1):
                    if t > 0:
                        zT = zpp.tile([128, 512], f32, tag="z", name="zT")
                        for k in range(4):
                            O = 8 * g + 4 * s + k
                            nc.tensor.matmul(
                                zT[:, k * 128:(k + 1) * 128],
                                W1d[:, O * 128:(O + 1) * 128], synT_sb[s][:],
                                start=(k == 0), stop=(k == 3),
                                skip_group_check=True)
                        nc.scalar.activation(
                            gT[s][:, g * 512:(g + 1) * 512], zT[:],
                            AF.Gelu, bias=b1v[:, 0:1])
                    if g == 0:
                        sp = xs0p[s][:]
                        vstart = (t == 0)
                        vstop = (t == STEPS - 1)
                    else:
                        spt = spp.tile([128, 1024], f32, tag="sp", name="sp")
                        sp = spt[:]
                        vstart, vstop = True, False
                    for bk in (0, 1):
                        nc.tensor.matmul(
                            sp[:, bk * 512:(bk + 1) * 512],
                            identS[:, t * 128:(t + 1) * 128],
                            V_sb[s][:, g * 1024 + bk * 512:
                                    g * 1024 + (bk + 1) * 512],
                            start=vstart, stop=False,
                            skip_group_check=True)
                    for k in range(4):
                        if t == 0:
                            break
                        nc.tensor.matmul(
                            sp[:, k * 256:(k + 1) * 256],
                            gT[s][:, (g * 4 + k) * 128:(g * 4 + k + 1) * 128],
                            bdcs[:, t * 256:(t + 1) * 256],
                            start=False,
                            stop=(vstop and k % 2 == 1) if g == 0
                            else (k % 2 == 1),
                            skip_group_check=True)
                    if g == 1:
                        nc.vector.tensor_tensor(
                            xs[s][:, 1024:2048], xs[s][:, 1024:2048],
                            sp[:], op=OP.add)

            # prologue (independent of u): overlap with Phase A DMA
            emit_norm(0, 0)
            emit_mid(0, 0)
            emit_norm(1, 0)
            emit_mid(1, 0)

            # ---------- Phase A: transposed u load + u_proj -> V ----------
            if True:
                for ch in range(8):
                    nc.sync.dma_start_transpose(
                        uT[:, ch * 2048:(ch + 1) * 2048],
                        u_t[ch * 2048:(ch + 1) * 2048, :])
                for q in (0, 1, 4, 5, 2, 3, 6, 7):  # stream-0 banks first
                    vp = spp.tile([128, 512], f32, tag="sp", name=f"vp{q}")
                    for k in range(16):
                        fc = 16 * q + k
                        nhi, b = fc // 64, fc % 64
                        r0 = b * 256 + nhi * 128
                        nc.tensor.matmul(
                            vp[:, k * 32:(k + 1) * 32],
                            uT[:, r0:r0 + 128], winTc[:],
                            start=(k % 16 == 0), stop=(k % 16 == 15),
                            skip_group_check=True)
                    nhi = (16 * q) // 64
                    bsl = baddSm[:, nhi * 32:(nhi + 1) * 32]
                    brd = bsl.unsqueeze(1).broadcast_to((128, 16, 32))
                    st, g, sub = (q // 2) % 2, q // 4, (q % 2) * 512
                    nc.vector.tensor_tensor(
                        V_sb[st][:, g * 1024 + sub:g * 1024 + sub + 512]
                        .rearrange("p (s d) -> p s d", d=32),
                        vp[:].rearrange("p (s d) -> p s d", d=32),
                        brd, op=OP.add)


            for t in range(STEPS):
                emit_chunks(0, t)
                if t > 0:
                    emit_norm(1, t)
                    emit_mid(1, t)
                if t < STEPS - 1:
                    emit_norm(0, t + 1)
                emit_chunks(1, t)
                if t < STEPS - 1:
                    emit_mid(0, t + 1)

            # drain PSUM-resident halves into xs for output
            for s in (0, 1):
                nc.scalar.copy(xs[s][:, 0:1024], xs0p[s][:])

            # ---------- Phase C: output ----------
            nc.sync.dma_start(xout[:, 0:2048], xs[0][:])
            nc.sync.dma_start(xout[:, 2048:4096], xs[1][:])
    nc.finalize()
    return nc


def _get_nc(n_cores):
    if n_cores not in _cache:
        _cache[n_cores] = build_nc(n_cores)
    return _cache[n_cores]


def kernel(u, features, bias, w_in, b_in, sig_w1, sig_b1, sig_w2, sig_b2):
    from concourse.bass_utils import run_bass_kernel_spmd

    u = np.asarray(u, np.float32)
    args = [np.asarray(a, np.float32) for a in
            (features, bias, w_in, b_in, sig_w1, sig_b1, sig_w2, sig_b2)]
    wt, winTc, baddSm, W1d, b1v, bdcs, identS = _host_prep(*args)

    nc = _get_nc(NCORES)

    in_maps = []
    for c in range(NCORES):
        u_shard = np.ascontiguousarray(
            u[c * BS:(c + 1) * BS].reshape(R, UIN)).astype(np.float16)
        in_maps.append({
            "u_t": u_shard, "wt": wt, "winTc": winTc, "baddSm": baddSm,
            "W1d": W1d, "b1v": b1v, "bdcs": bdcs, "identS": identS,
        })
    res = run_bass_kernel_spmd(nc, in_maps, list(range(NCORES)))

    scale = np.float32(C1 ** STEPS)
    out = np.empty((B, N, D), np.float32)
    for c in range(NCORES):
        xo = res.results[c]["xout"].astype(np.float32) * scale  # [128, 4096]
        # xo[nlo, s*2048 + h*1024 + fcs*32 + d]; b = s*32+fcs, n = h*128+nlo
        v = xo.reshape(128, 2, 2, 32, 32)            # [nlo, s, h, fcs, d]
        out[c * BS:(c + 1) * BS] = (
            v.transpose(1, 3, 2, 0, 4).reshape(BS, N, D))
    return out
